# revision 2
# baseline (speedup 1.0000x reference)
"""BitDense (binary dense layer) Trainium2 kernel.

Computation (for the full problem):
    inputs: [1024] uint32   packed input bits (32768 bits)
    w:      [32768, 1024]   packed weight bits per unit
    b:      [32768] int32   bias
    ones[u]   = sum_k popcount(inputs[k] ^ w[u,k])
    out_i[u]  = 32768 - 2*ones[u] + b[u]
    bool[u]   = out_i[u] < 0
    output    = packbits(bool) -> [1024] uint32   (MSB-first per byte,
                little-endian bytes per word)

Sharding: w/b row-sharded over units across 8 NeuronCores (4096 units
each). Each core computes its ones[] slice with a DVE SWAR popcount:
bitwise steps on uint32 lanes (tensor_scalar ops hit the 2x_2p DVE perf
mode), the three adds on uint16 views (DVE arithmetic is fp32-internal,
so values must stay < 2^24; u16 lanes max at 65535 — exact). The final
per-byte counts are reduced by the otherwise-idle ScalarE (ACT) via a
uint8 view: sum of all bytes of g IS the popcount sum. The packed-input
vector is broadcast (host-replicated to 128 partitions). Threshold +
bit-packing of the 1024 output words is done on host (32768 bools).

Tiles are processed as [128, 2*K] "super-tiles" (two 128-unit row
blocks per DVE instruction, 1 MiB DMAs) to halve per-instruction
overhead; ACT accumulates each row block separately.
"""

import sys

for _p in ("/opt/trn_rl_repo",):
    if _p not in sys.path:
        sys.path.insert(0, _p)

import numpy as np

import concourse.bass as bass
import concourse.bacc as bacc
import concourse.mybir as mybir
from concourse.tile import TileContext
from concourse.bass_utils import run_bass_kernel_spmd

A = mybir.AluOpType
DT = mybir.dt

N_CORES = 8
REPEAT = 1                    # timing experiments only; must be 1 for grading
UNITS = 32768
K = 1024                      # packed input words per unit
UPC = UNITS // N_CORES        # units per core = 4096
P = 128                       # SBUF partitions
BLOCKS = UPC // P             # 128-unit row blocks per core = 32
SUPER = 2                     # row blocks per super-tile
TILES = BLOCKS // SUPER       # super-tiles per core = 16
W = SUPER * K                 # words per super-tile row = 2048


def _build_program():
    """One SPMD Bass program: per-core ones[] counts for a [UPC, K] w slice."""
    nc = bacc.Bacc("TRN2", target_bir_lowering=False)
    w_d = nc.dram_tensor("w", [UPC, K], DT.uint32, kind="ExternalInput")
    x_d = nc.dram_tensor("xrep", [P, K], DT.uint32, kind="ExternalInput")
    o_d = nc.dram_tensor("cnt", [P, BLOCKS], DT.float32, kind="ExternalOutput")

    with TileContext(nc) as tc:
        with tc.tile_pool(name="wp", bufs=3) as wp, \
             tc.tile_pool(name="xp", bufs=1) as xp, \
             tc.tile_pool(name="sp", bufs=2) as sp, \
             tc.tile_pool(name="ac", bufs=1) as ac:
            xr = xp.tile([P, K], DT.uint32, tag="xr")
            nc.sync.dma_start(out=xr[:], in_=x_d[:, :])

            ones_c = ac.tile([P, BLOCKS], DT.float32, tag="ones")
            hi_c = ac.tile([P, BLOCKS], DT.float32, tag="hi")
            nc.vector.memset(hi_c[:], 0.0)

            for t in range(TILES):
                wt = wp.tile([P, W], DT.uint32, tag="wt")
                # super-tile: row blocks 2t and 2t+1 side by side in the
                # free dim; DRAM rows 128*(2t)+p -> cols [0,K),
                #            rows 128*(2t+1)+p -> cols [K,2K)
                for s in range(SUPER):
                    blk = SUPER * t + s
                    nc.sync.dma_start(
                        out=wt[:, s * K:(s + 1) * K],
                        in_=w_d[P * blk:P * (blk + 1), :])
                # y = w ^ x  (in place; x repeats for both blocks)
                for s in range(SUPER):
                    nc.vector.tensor_tensor(out=wt[:, s * K:(s + 1) * K],
                                            in0=wt[:, s * K:(s + 1) * K],
                                            in1=xr[:], op=A.bitwise_xor)
                for _r in range(REPEAT):
                    a = sp.tile([P, W], DT.uint32, tag="a")
                    c2 = sp.tile([P, W], DT.uint32, tag="c2")
                    a16 = a[:].bitcast(DT.uint16)
                    c16 = c2[:].bitcast(DT.uint16)
                    y16 = wt[:].bitcast(DT.uint16)
                    # a = (y >> 1) & 0x55555555
                    nc.vector.tensor_scalar(out=a[:], in0=wt[:], scalar1=1,
                                            scalar2=0x55555555,
                                            op0=A.logical_shift_right,
                                            op1=A.bitwise_and)
                    # a <- b = y - a        (pairwise 2-bit counts; u16 lanes)
                    nc.vector.tensor_tensor(out=a16, in0=y16, in1=a16,
                                            op=A.subtract)
                    # c2 = (b >> 2) & 0x33333333
                    nc.vector.tensor_scalar(out=c2[:], in0=a[:], scalar1=2,
                                            scalar2=0x33333333,
                                            op0=A.logical_shift_right,
                                            op1=A.bitwise_and)
                    # a <- d = b & 0x33333333   (GpSimd, overlaps DVE)
                    nc.vector.tensor_scalar(out=a[:], in0=a[:],
                                            scalar1=0x33333333,
                                            scalar2=None, op0=A.bitwise_and)
                    # c2 <- e = c + d       (nibble counts; u16 lanes)
                    nc.vector.tensor_tensor(out=c16, in0=c16, in1=a16,
                                            op=A.add)
                    # hybrid tail: block 0 classic DVE tail (e4/f/g + 1 ACT
                    # reduce), block 1 ACT-heavy tail (h-mask + 2 ACT reduces)
                    dump = sp.tile([P, 4 * K], DT.uint8, tag="dump")
                    # block 0 classic:
                    nc.vector.tensor_scalar(out=a[:, 0:K], in0=c2[:, 0:K],
                                            scalar1=4, scalar2=None,
                                            op0=A.logical_shift_right)
                    nc.vector.tensor_tensor(out=a16[:, 0:2*K], in0=c16[:, 0:2*K],
                                            in1=a16[:, 0:2*K], op=A.add)
                    nc.vector.tensor_scalar(out=c2[:, 0:K], in0=a[:, 0:K],
                                            scalar1=0x0F0F0F0F,
                                            scalar2=None, op0=A.bitwise_and)
                    g8 = c2[:].bitcast(DT.uint8)
                    blk = SUPER * t
                    nc.scalar.activation(out=dump[:], in_=g8[:, 0:4*K],
                                         func=mybir.ActivationFunctionType.Copy,
                                         accum_out=ones_c[:, blk:blk + 1])
                    # block 1 ACT-tail: h = (e>>4)&0x0F0F0F0F; sum = Se8-15*Sh8
                    nc.vector.tensor_scalar(out=a[:, K:2*K], in0=c2[:, K:2*K],
                                            scalar1=4, scalar2=0x0F0F0F0F,
                                            op0=A.logical_shift_right,
                                            op1=A.bitwise_and)
                    e8 = c2[:].bitcast(DT.uint8)
                    h8 = a[:].bitcast(DT.uint8)
                    nc.scalar.activation(out=dump[:], in_=e8[:, 4*K:8*K],
                                         func=mybir.ActivationFunctionType.Copy,
                                         accum_out=ones_c[:, blk+1:blk+2])
                    nc.scalar.activation(out=dump[:], in_=h8[:, 4*K:8*K],
                                         func=mybir.ActivationFunctionType.Copy,
                                         accum_out=hi_c[:, blk+1:blk+2])

            # odd blocks: counts = ones - 15*hi (hi is zero for even blocks
            # only if never accumulated -- it is memset-free, so subtract a
            # zeroed product only where written; simplest: always subtract,
            # hi_c columns for even blocks are never written -> must zero them
            cnt_f = ac.tile([P, BLOCKS], DT.float32, tag="cf")
            nc.vector.tensor_scalar(out=cnt_f[:], in0=hi_c[:], scalar1=15.0,
                                    scalar2=None, op0=A.mult)
            nc.vector.tensor_tensor(out=cnt_f[:], in0=ones_c[:], in1=cnt_f[:],
                                    op=A.subtract)
            nc.sync.dma_start(out=o_d[:, :], in_=cnt_f[:])
    nc.finalize()
    return nc


_NC_CACHE = None

# test.py sets TRACE=True to capture an NTFF profile; LAST_EXEC_NS /
# LAST_TRACE then hold the most recent hardware timing. The graded path
# leaves TRACE=False.
TRACE = False
LAST_EXEC_NS = None
LAST_TRACE = None


def _get_program():
    global _NC_CACHE
    if _NC_CACHE is None:
        _NC_CACHE = _build_program()
    return _NC_CACHE


def _make_in_maps(inp):
    inputs = np.asarray(inp["inputs"]).view(np.uint32).reshape(K)
    w = np.asarray(inp["w"]).view(np.uint32).reshape(UNITS, K)
    xrep = np.ascontiguousarray(np.broadcast_to(inputs[None, :], (P, K)))
    return [
        {
            "w": np.ascontiguousarray(w[c * UPC:(c + 1) * UPC]),
            "xrep": xrep,
        }
        for c in range(N_CORES)
    ]


def kernel(inputs, w, b):
    b = np.asarray(b).view(np.int32).reshape(UNITS)
    in_maps = _make_in_maps({"inputs": inputs, "w": w})

    nc = _get_program()
    res = run_bass_kernel_spmd(nc, in_maps, core_ids=list(range(N_CORES)),
                               trace=TRACE)
    if TRACE:
        global LAST_EXEC_NS, LAST_TRACE
        LAST_EXEC_NS = res.exec_time_ns
        LAST_TRACE = res

    ones = np.empty(UNITS, dtype=np.int64)
    for c in range(N_CORES):
        cnt = np.asarray(res.results[c]["cnt"])        # [P, BLOCKS] fp32
        ones[c * UPC:(c + 1) * UPC] = cnt.T.reshape(UPC).astype(np.int64)

    out_i = 32768 - 2 * ones + b.astype(np.int64)
    bools = out_i < 0
    packed = np.packbits(bools).view(np.uint32)        # [1024]
    return packed

